# revision 22
# baseline (speedup 1.0000x reference)
"""Trainium2 Bass kernel for SimCLR NT-Xent contrastive loss (N=4096, D=512, T=0.5).

Math: with z = rownorm(concat(emb_i, emb_j)) (8192x512) and S = z @ z.T:
  loss = (1/2N) * [ sum_r log(rowsum_r(exp(S/T)) - exp(1/T)) - 2*sum_i log e_pos_i ]

v2 design (vs v1 full-grid bf16):
  * fp8(e4m3) z with x16 prescale; PE matmuls in DoubleRow perf mode
    (2 k-tiles per instruction, 0.5 cycles/row).
  * Symmetric circulant-band decomposition: S is symmetric, so each
    128-row tile m computes only columns [m, m+32] (mod 64 tiles); exp'd
    blocks contribute row-sums (ACT accum) AND column-sums (ones-vector
    matmul on PE). Halves the exp work - ACT is the serial bottleneck.
  * fp8 z written to DRAM, xbar-transposed as uint16 PAIRS: the 2-byte
    transpose of adjacent-d fp8 pairs lands exactly in the DoubleRow
    ifmap layout [p, 2, n] with k(p,i) = 2p+i.
  * Core c gets np.roll(z_input, -128*c): its 8 m-tiles are at fixed
    local positions 1024*t; bands wrap into a +3200-col extension of zT.
  * Partial row/col denominator sums + positive-pair diag exp'd values
    are DMA'd out; host does the cross-core scatter-add, log, and final
    reduction (cheap: ~100KB/core).
"""

import numpy as np

for _p in ("/opt/trn_rl_repo", "/root/.axon_site/_ro/trn_rl_repo"):
    try:
        import concourse  # noqa: F401
        break
    except ImportError:
        import sys
        if _p not in sys.path:
            sys.path.insert(0, _p)

import concourse.bass as bass
import concourse.bacc as bacc
import concourse.tile as tile
from concourse import mybir
from concourse.bass_utils import run_bass_kernel_spmd

F32 = mybir.dt.float32
I32 = mybir.dt.int32
U16 = mybir.dt.uint16
BF16 = mybir.dt.bfloat16
F8 = mybir.dt.float8e4
ALU = mybir.AluOpType
AF = mybir.ActivationFunctionType
DR = mybir.MatmulPerfMode.DoubleRowSwInterleave

N_CORES = 8
BATCH = 4096
DIM = 512
ROWS = 2 * BATCH            # 8192
P = 128                     # partitions
NT = ROWS // P              # 64 row tiles
NG = 16                     # load groups (4 row-tiles each)
TPG = 4                     # tiles per group
RG = 4                      # DRAM scratch row-ranges (2048 rows each)
EXT = 3200                  # zT column extension (25 tiles) for band wrap
ZTW = ROWS + EXT            # 11392 zT columns
PRE = 16.0                  # fp8 prescale; S_psum = 256 * s
EXP_SCALE = 2.0 / (PRE * PRE)   # exp((1/T) * s) with T=0.5
MAGIC = 0x5F3759DF
CPW = 4224                  # colsum partial row width per strip


def _build_program():
    nc = bacc.Bacc(trn_type="TRN2")
    x_in = nc.declare_dram_parameter("x", [ROWS, DIM], F32, isOutput=False)
    rowp_out = nc.declare_dram_parameter("rowp", [P, 8], F32, isOutput=True)
    posd_out = nc.declare_dram_parameter("posd", [P, 4], F32, isOutput=True)
    colp_out = nc.declare_dram_parameter("colp", [8, CPW], F32, isOutput=True)

    with tile.TileContext(nc) as tc:
        with tc.tile_pool(name="xg", bufs=3) as xg_pool, \
             tc.tile_pool(name="zbig", bufs=2) as zbig_pool, \
             tc.tile_pool(name="sq", bufs=2) as sq_pool, \
             tc.tile_pool(name="small", bufs=2) as small_pool, \
             tc.tile_pool(name="single", bufs=1) as singles, \
             tc.tile_pool(name="escr", bufs=3) as e_pool, \
             tc.tile_pool(name="e32", bufs=2) as e32_pool, \
             tc.tile_pool(name="cstage", bufs=2) as cst_pool, \
             tc.tile_pool(name="zdram", bufs=1, space="DRAM") as dram_pool, \
             tc.tile_pool(name="psS", bufs=2, space="PSUM") as ps_pool, \
             tc.tile_pool(name="psC", bufs=2, space="PSUM") as cs_pool:

            n2 = singles.tile([P, NT], F32, tag="n2")
            inv = singles.tile([P, NT], F32, tag="inv")
            magic4 = singles.tile([P, TPG], I32, tag="magic4")
            nc.vector.memset(magic4, MAGIC)
            ones8 = singles.tile([P, 1], F8, tag="ones8")
            nc.vector.memset(ones8, 1.0)
            ones16 = singles.tile([P, 1], BF16, tag="ones16")
            nc.vector.memset(ones16, 1.0)
            rowacc = singles.tile([P, 5], F32, tag="rowacc")
            rowp_sb = singles.tile([P, 8], F32, tag="rowp_sb")
            posd_sb = singles.tile([P, 4], F32, tag="posd_sb")

            # fp8 z scratch in DRAM, 4 ranges of 2048 rows
            zd = [dram_pool.tile([ROWS // RG, DIM], F8, tag=f"zd{r}", name=f"zd{r}")
                  for r in range(RG)]
            # zT[ki]: [128, ZTW] uint16; cell (p, r) = fp8 pair
            # (z[r, 256ki+2p], z[r, 256ki+2p+1])
            zT = [singles.tile([P, ZTW], U16, tag=f"zt{k}", name=f"zt{k}")
                  for k in range(2)]

            def dr_view(ki, col0, w):
                # DoubleRow ifmap [128, 2, w]: k(p,i) = 256*ki + 2p + i
                return zT[ki].bitcast(F8)[:, 2 * col0: 2 * (col0 + w)].rearrange(
                    "p (n two) -> p two n", two=2)

            def dr_lhsT(ki, col0):
                # DoubleRowSwInterleave weights [128, m-reversed, 2]: the
                # byte-interleaved A/B pairs with columns last-first, as the
                # dual-fp8 ldweights path expects
                v = zT[ki].bitcast(F8).rearrange("p (n two) -> p n two", two=2)
                if col0 == 0:
                    return v[:, P - 1::-1, :]
                return v[:, col0 + P - 1:col0 - 1:-1, :]

            zbigs = {}

            # ---- Phase 1: normalize rows -> fp8 z (x16) -> DRAM scratch ----
            for g in range(NG):
                r0 = g * TPG * P
                xg = xg_pool.tile([P, TPG, DIM], F32, tag="xg")
                nc.sync.dma_start(
                    out=xg,
                    in_=x_in[r0:r0 + TPG * P, :].rearrange("(a p) d -> p a d", p=P))
                for a in range(TPG):
                    sq = sq_pool.tile([P, DIM], F32, tag="sq")
                    nc.vector.scalar_tensor_tensor(
                        out=sq, in0=xg[:, a, :], scalar=0.0, in1=xg[:, a, :],
                        op0=ALU.bypass, op1=ALU.mult,
                        accum_out=n2[:, g * TPG + a: g * TPG + a + 1])
                # rsqrt via Quake seed + 2 Newton steps, then x16 prescale
                sl = n2[:, g * TPG:(g + 1) * TPG]
                isl = inv[:, g * TPG:(g + 1) * TPG]
                sh = small_pool.tile([P, TPG], I32, tag="sh")
                nc.vector.tensor_scalar(
                    out=sh, in0=sl.bitcast(I32), scalar1=1, scalar2=None,
                    op0=ALU.logical_shift_right)
                seed = small_pool.tile([P, TPG], I32, tag="seed")
                nc.vector.scalar_tensor_tensor(
                    out=seed, in0=magic4, scalar=0.0, in1=sh,
                    op0=ALU.bypass, op1=ALU.subtract)
                y = seed.bitcast(F32)
                for it in range(2):
                    ta = small_pool.tile([P, TPG], F32, tag="ta")
                    tb = small_pool.tile([P, TPG], F32, tag="tb")
                    nc.vector.tensor_mul(out=ta, in0=y, in1=y)
                    nc.vector.scalar_tensor_tensor(
                        out=tb, in0=ta, scalar=-0.5, in1=sl,
                        op0=ALU.mult, op1=ALU.mult)
                    nc.vector.tensor_scalar(
                        out=tb, in0=tb, scalar1=1.5, scalar2=None, op0=ALU.add)
                    dst = isl if it == 1 else y
                    nc.vector.tensor_mul(out=dst, in0=y, in1=tb)
                nc.vector.tensor_scalar(
                    out=isl, in0=isl, scalar1=PRE, scalar2=None, op0=ALU.mult)

                rr = g // 4
                if g % 4 == 0:
                    zbigs[rr] = zbig_pool.tile(
                        [P, 4 * TPG, DIM], F8, tag="zbig", name=f"zbig{rr}")
                zb = zbigs[rr]
                jlo = (g % 4) * TPG
                for a in range(TPG):
                    sc = inv[:, g * TPG + a: g * TPG + a + 1]
                    # split scale+fp8-cast across DVE / ACT / GpSimd
                    if a == 0:
                        nc.vector.tensor_scalar_mul(
                            out=zb[:, jlo + a, :], in0=xg[:, a, :], scalar1=sc)
                    elif a == 1:
                        nc.scalar.mul(zb[:, jlo + a, :], xg[:, a, :], sc)
                    else:
                        nc.gpsimd.tensor_scalar_mul(
                            out=zb[:, jlo + a, :], in0=xg[:, a, :], scalar1=sc)
                if g % 4 == 3:
                    nc.sync.dma_start(
                        out=zd[rr][:, :].rearrange("(s p) d -> p s d", p=P),
                        in_=zb)

            # ---- Phase 2: xbar transpose fp8 pairs as uint16 -> zT ----
            for rr in range(RG):
                zdu = zd[rr].bitcast(U16)     # [2048, 256]
                for ki in range(2):
                    nc.sync.dma_start_transpose(
                        out=zT[ki][:, rr * 2048:(rr + 1) * 2048],
                        in_=zdu[:, ki * P:(ki + 1) * P])
            # extension: local cols [8192, 11392) = rows [0, 3200)
            for ki in range(2):
                nc.sync.dma_start_transpose(
                    out=zT[ki][:, ROWS:ROWS + 2048],
                    in_=zd[0].bitcast(U16)[:, ki * P:(ki + 1) * P])
                nc.sync.dma_start_transpose(
                    out=zT[ki][:, ROWS + 2048:ZTW],
                    in_=zd[1].bitcast(U16)[0:EXT - 2048, ki * P:(ki + 1) * P])

            # ---- Phase 3: banded symmetric S blocks ----
            for t in range(8):
                mlo = 1024 * t
                lhsT = [dr_lhsT(ki, mlo) for ki in range(2)]
                # colsum PSUM tiles: blocks land at base partitions
                # 0/32/64/96 so one strided copy drains 4 blocks at once
                csA = cs_pool.tile([P, 1024], F32, tag="csA")
                stage = cst_pool.tile([1, CPW], F32, tag="cstage")
                nc.vector.memset(stage[0:1, 0:P], 0.0)
                for pair in range(4):
                    ps = ps_pool.tile([P, 1024], F32, tag="ps")
                    for ki in range(2):
                        for b2 in range(2):
                            c0 = mlo + 1024 * pair + 512 * b2
                            nc.tensor.matmul(
                                ps[:, 512 * b2:512 * (b2 + 1)],
                                lhsT=lhsT[ki], rhs=dr_view(ki, c0, 512),
                                start=(ki == 0), stop=(ki == 1), perf_mode=DR)
                    e_scr = e_pool.tile([P, 1024], F8, tag="escr")
                    nc.scalar.activation(
                        out=e_scr, in_=ps, func=AF.Exp, scale=EXP_SCALE,
                        accum_out=rowacc[:, pair:pair + 1])
                    for b2 in range(2):
                        blk = 2 * pair + b2          # 0..7
                        # quadrant row 32*(blk//2) holds blocks 2k, 2k+1
                        co = 512 * (blk % 2)
                        bp = 32 * (blk // 2)
                        lo = P if blk == 0 else 0
                        nc.tensor.matmul(
                            csA[bp:bp + 1, co + lo:co + 512],
                            lhsT=ones8[:, 0:1],
                            rhs=e_scr[:, 512 * b2 + lo:512 * (b2 + 1)],
                            start=True, stop=True, tile_position=(0, bp))
                # drain colsums: one copy per PE quadrant row (DVE+ACT split)
                nc.vector.tensor_scalar_add(
                    out=stage[0:1, P:1024], in0=csA[0:1, P:1024], scalar1=0.0)
                nc.vector.tensor_scalar_add(
                    out=stage[0:1, 1024:2048], in0=csA[32:33, 0:1024], scalar1=0.0)
                nc.scalar.copy(out=stage[0:1, 2048:3072], in_=csA[64:65, 0:1024])
                nc.scalar.copy(out=stage[0:1, 3072:4096], in_=csA[96:97, 0:1024])
                if t < 4:
                    ps = ps_pool.tile([P, 1024], F32, tag="ps")
                    for ki in range(2):
                        nc.tensor.matmul(
                            ps[:, 0:P], lhsT=lhsT[ki],
                            rhs=dr_view(ki, mlo + 4096, P),
                            start=(ki == 0), stop=(ki == 1), perf_mode=DR)
                    e32 = e32_pool.tile([P, P], BF16, tag="e32")
                    nc.scalar.activation(
                        out=e32, in_=ps[:, 0:P], func=AF.Exp, scale=EXP_SCALE,
                        accum_out=rowacc[:, 4:5])
                    # j32 colsum reuses the unused half of the j32 S psum tile
                    nc.tensor.matmul(
                        ps[0:1, 512:512 + P], lhsT=ones16[:, 0:1], rhs=e32,
                        start=True, stop=True)
                    nc.vector.tensor_scalar_add(
                        out=stage[0:1, 4096:CPW], in0=ps[0:1, 512:512 + P],
                        scalar1=0.0)
                    dsel = e32_pool.tile([P, P], BF16, tag="dsel")
                    nc.gpsimd.affine_select(
                        out=dsel, in_=e32, pattern=[[1, P]],
                        compare_op=ALU.is_equal, fill=0.0,
                        base=0, channel_multiplier=-1)
                    nc.vector.reduce_sum(
                        out=posd_sb[:, t:t + 1], in_=dsel,
                        axis=mybir.AxisListType.X)
                else:
                    nc.vector.memset(rowacc[:, 4:5], 0.0)
                nc.vector.reduce_sum(
                    out=rowp_sb[:, t:t + 1], in_=rowacc,
                    axis=mybir.AxisListType.X)
                kmax = CPW if t < 4 else 4096
                nc.sync.dma_start(
                    out=colp_out[t:t + 1, 0:kmax], in_=stage[0:1, 0:kmax])

            nc.sync.dma_start(out=rowp_out[:, :], in_=rowp_sb)
            nc.sync.dma_start(out=posd_out[:, :], in_=posd_sb)

    nc.finalize()
    return nc


_CACHE = {}


def _run(full: np.ndarray, trace: bool = False, **kwargs):
    """Run the SPMD program on all 8 cores; returns BassKernelResults."""
    if "nc" not in _CACHE:
        _CACHE["nc"] = _build_program()
    nc = _CACHE["nc"]
    in_maps = [
        {"x": np.ascontiguousarray(np.roll(full, -P * c, axis=0))}
        for c in range(N_CORES)
    ]
    return run_bass_kernel_spmd(
        nc, in_maps, core_ids=list(range(N_CORES)), trace=trace, **kwargs)


def _merge(results) -> np.ndarray:
    denom = np.zeros(ROWS, np.float64)
    pos = np.zeros(BATCH, np.float64)
    idx = np.arange(P)
    for c, r in enumerate(results):
        rowp = r["rowp"].astype(np.float64)
        colp = r["colp"].astype(np.float64)
        posd = r["posd"].astype(np.float64)
        for t in range(8):
            gr = (1024 * t + idx + P * c) % ROWS
            denom[gr] += rowp[:, t]
            kmax = CPW if t < 4 else 4096
            k = np.arange(P, kmax)
            gc = (1024 * t + k + P * c) % ROWS
            np.add.at(denom, gc, colp[t, k])
            if t < 4:
                pos[(1024 * t + idx + P * c) % ROWS] = posd[:, t]
    denom -= np.exp(2.0)
    loss = (np.log(denom).sum() - 2.0 * np.log(pos).sum()) / ROWS
    return np.array(loss, dtype=np.float32)


def kernel(emb_i: np.ndarray, emb_j: np.ndarray) -> np.ndarray:
    full = np.concatenate(
        [np.asarray(emb_i, np.float32), np.asarray(emb_j, np.float32)], axis=0)
    return _merge(_run(full).results)


# revision 23
# speedup vs baseline: 1.9001x; 1.9001x over previous
"""Trainium2 Bass kernel for SimCLR NT-Xent contrastive loss (N=4096, D=512, T=0.5).

Math: with z = rownorm(concat(emb_i, emb_j)) (8192x512) and S = z @ z.T:
  loss = (1/2N) * [ sum_r log(rowsum_r(exp(S/T)) - exp(1/T)) - 2*sum_i log e_pos_i ]

v2 design (vs v1 full-grid bf16):
  * fp8(e4m3) z with x16 prescale; PE matmuls in DoubleRow perf mode
    (2 k-tiles per instruction, 0.5 cycles/row).
  * Symmetric circulant-band decomposition: S is symmetric, so each
    128-row tile m computes only columns [m, m+32] (mod 64 tiles); exp'd
    blocks contribute row-sums (ACT accum) AND column-sums (ones-vector
    matmul on PE). Halves the exp work - ACT is the serial bottleneck.
  * fp8 z written to DRAM, xbar-transposed as uint16 PAIRS: the 2-byte
    transpose of adjacent-d fp8 pairs lands exactly in the DoubleRow
    ifmap layout [p, 2, n] with k(p,i) = 2p+i.
  * Core c gets np.roll(z_input, -128*c): its 8 m-tiles are at fixed
    local positions 1024*t; bands wrap into a +3200-col extension of zT.
  * Partial row/col denominator sums + positive-pair diag exp'd values
    are DMA'd out; host does the cross-core scatter-add, log, and final
    reduction (cheap: ~100KB/core).
"""

import numpy as np

for _p in ("/opt/trn_rl_repo", "/root/.axon_site/_ro/trn_rl_repo"):
    try:
        import concourse  # noqa: F401
        break
    except ImportError:
        import sys
        if _p not in sys.path:
            sys.path.insert(0, _p)

import concourse.bass as bass
import concourse.bacc as bacc
import concourse.tile as tile
from concourse import mybir
from concourse.bass_utils import run_bass_kernel_spmd

F32 = mybir.dt.float32
I32 = mybir.dt.int32
U16 = mybir.dt.uint16
BF16 = mybir.dt.bfloat16
F8 = mybir.dt.float8e4
ALU = mybir.AluOpType
AF = mybir.ActivationFunctionType
DR = mybir.MatmulPerfMode.DoubleRowSwInterleave

N_CORES = 8
BATCH = 4096
DIM = 512
ROWS = 2 * BATCH            # 8192
P = 128                     # partitions
NT = ROWS // P              # 64 row tiles
NG = 16                     # load groups (4 row-tiles each)
TPG = 4                     # tiles per group
RG = 4                      # DRAM scratch row-ranges (2048 rows each)
EXT = 3200                  # zT column extension (25 tiles) for band wrap
ZTW = ROWS + EXT            # 11392 zT columns
PRE = 16.0                  # fp8 prescale; S_psum = 256 * s
EXP_SCALE = 2.0 / (PRE * PRE)   # exp((1/T) * s) with T=0.5
MAGIC = 0x5F3759DF
CPW = 4224                  # colsum partial row width per strip


def _build_program():
    nc = bacc.Bacc(trn_type="TRN2")
    x_in = nc.declare_dram_parameter("x", [ROWS, DIM], F32, isOutput=False)
    rowp_out = nc.declare_dram_parameter("rowp", [P, 8], F32, isOutput=True)
    posd_out = nc.declare_dram_parameter("posd", [P, 4], F32, isOutput=True)
    colp_out = nc.declare_dram_parameter("colp", [8, CPW], F32, isOutput=True)

    with tile.TileContext(nc) as tc:
        with tc.tile_pool(name="xg", bufs=3) as xg_pool, \
             tc.tile_pool(name="zbig", bufs=2) as zbig_pool, \
             tc.tile_pool(name="sq", bufs=2) as sq_pool, \
             tc.tile_pool(name="small", bufs=2) as small_pool, \
             tc.tile_pool(name="single", bufs=1) as singles, \
             tc.tile_pool(name="escr", bufs=3) as e_pool, \
             tc.tile_pool(name="e32", bufs=2) as e32_pool, \
             tc.tile_pool(name="cstage", bufs=2) as cst_pool, \
             tc.tile_pool(name="zdram", bufs=1, space="DRAM") as dram_pool, \
             tc.tile_pool(name="psS", bufs=2, space="PSUM") as ps_pool, \
             tc.tile_pool(name="psC", bufs=2, space="PSUM") as cs_pool:

            n2 = singles.tile([P, NT], F32, tag="n2")
            inv = singles.tile([P, NT], F32, tag="inv")
            magic4 = singles.tile([P, TPG], I32, tag="magic4")
            nc.vector.memset(magic4, MAGIC)
            ones8 = singles.tile([P, 1], F8, tag="ones8")
            nc.vector.memset(ones8, 1.0)
            ones16 = singles.tile([P, 1], BF16, tag="ones16")
            nc.vector.memset(ones16, 1.0)
            rowacc = singles.tile([P, 5], F32, tag="rowacc")
            rowp_sb = singles.tile([P, 8], F32, tag="rowp_sb")
            posd_sb = singles.tile([P, 4], F32, tag="posd_sb")

            # fp8 z scratch in DRAM, 4 ranges of 2048 rows
            zd = [dram_pool.tile([ROWS // RG, DIM], F8, tag=f"zd{r}", name=f"zd{r}")
                  for r in range(RG)]
            # zT[ki]: [128, ZTW] uint16; cell (p, r) = fp8 pair
            # (z[r, 256ki+2p], z[r, 256ki+2p+1])
            zT = [singles.tile([P, ZTW], U16, tag=f"zt{k}", name=f"zt{k}")
                  for k in range(2)]

            def dr_view(ki, col0, w):
                # DoubleRow ifmap [128, 2, w]: k(p,i) = 256*ki + 2p + i
                return zT[ki].bitcast(F8)[:, 2 * col0: 2 * (col0 + w)].rearrange(
                    "p (n two) -> p two n", two=2)

            def dr_lhsT(ki, col0):
                # DoubleRowSwInterleave weights [128, m-reversed, 2]: the
                # byte-interleaved A/B pairs with columns last-first, as the
                # dual-fp8 ldweights path expects
                v = zT[ki].bitcast(F8).rearrange("p (n two) -> p n two", two=2)
                if col0 == 0:
                    return v[:, P - 1::-1, :]
                return v[:, col0 + P - 1:col0 - 1:-1, :]

            zbigs = {}

            # ---- Phase 1: normalize rows -> fp8 z (x16) -> DRAM scratch ----
            for g in range(NG):
                r0 = g * TPG * P
                xg = xg_pool.tile([P, TPG, DIM], F32, tag="xg")
                nc.sync.dma_start(
                    out=xg,
                    in_=x_in[r0:r0 + TPG * P, :].rearrange("(a p) d -> p a d", p=P))
                for a in range(TPG):
                    sq = sq_pool.tile([P, DIM], F32, tag="sq")
                    nc.vector.scalar_tensor_tensor(
                        out=sq, in0=xg[:, a, :], scalar=0.0, in1=xg[:, a, :],
                        op0=ALU.bypass, op1=ALU.mult,
                        accum_out=n2[:, g * TPG + a: g * TPG + a + 1])
                # rsqrt via Quake seed + 2 Newton steps, then x16 prescale
                sl = n2[:, g * TPG:(g + 1) * TPG]
                isl = inv[:, g * TPG:(g + 1) * TPG]
                sh = small_pool.tile([P, TPG], I32, tag="sh")
                nc.vector.tensor_scalar(
                    out=sh, in0=sl.bitcast(I32), scalar1=1, scalar2=None,
                    op0=ALU.logical_shift_right)
                seed = small_pool.tile([P, TPG], I32, tag="seed")
                nc.vector.scalar_tensor_tensor(
                    out=seed, in0=magic4, scalar=0.0, in1=sh,
                    op0=ALU.bypass, op1=ALU.subtract)
                y = seed.bitcast(F32)
                for it in range(2):
                    ta = small_pool.tile([P, TPG], F32, tag="ta")
                    tb = small_pool.tile([P, TPG], F32, tag="tb")
                    nc.vector.tensor_mul(out=ta, in0=y, in1=y)
                    nc.vector.scalar_tensor_tensor(
                        out=tb, in0=ta, scalar=-0.5, in1=sl,
                        op0=ALU.mult, op1=ALU.mult)
                    nc.vector.tensor_scalar(
                        out=tb, in0=tb, scalar1=1.5, scalar2=None, op0=ALU.add)
                    dst = isl if it == 1 else y
                    nc.vector.tensor_mul(out=dst, in0=y, in1=tb)
                nc.vector.tensor_scalar(
                    out=isl, in0=isl, scalar1=PRE, scalar2=None, op0=ALU.mult)

                rr = g // 4
                if g % 4 == 0:
                    zbigs[rr] = zbig_pool.tile(
                        [P, 4 * TPG, DIM], F8, tag="zbig", name=f"zbig{rr}")
                zb = zbigs[rr]
                jlo = (g % 4) * TPG
                for a in range(TPG):
                    sc = inv[:, g * TPG + a: g * TPG + a + 1]
                    # split scale+fp8-cast across DVE / ACT
                    if a < 2:
                        nc.vector.tensor_scalar_mul(
                            out=zb[:, jlo + a, :], in0=xg[:, a, :], scalar1=sc)
                    else:
                        nc.scalar.mul(zb[:, jlo + a, :], xg[:, a, :], sc)
                if g % 4 == 3:
                    nc.sync.dma_start(
                        out=zd[rr][:, :].rearrange("(s p) d -> p s d", p=P),
                        in_=zb)

            # ---- Phase 2: xbar transpose fp8 pairs as uint16 -> zT ----
            for rr in range(RG):
                zdu = zd[rr].bitcast(U16)     # [2048, 256]
                for ki in range(2):
                    nc.sync.dma_start_transpose(
                        out=zT[ki][:, rr * 2048:(rr + 1) * 2048],
                        in_=zdu[:, ki * P:(ki + 1) * P])
            # extension: local cols [8192, 11392) = rows [0, 3200)
            for ki in range(2):
                nc.sync.dma_start_transpose(
                    out=zT[ki][:, ROWS:ROWS + 2048],
                    in_=zd[0].bitcast(U16)[:, ki * P:(ki + 1) * P])
                nc.sync.dma_start_transpose(
                    out=zT[ki][:, ROWS + 2048:ZTW],
                    in_=zd[1].bitcast(U16)[0:EXT - 2048, ki * P:(ki + 1) * P])

            # ---- Phase 3: banded symmetric S blocks ----
            for t in range(8):
                mlo = 1024 * t
                lhsT = [dr_lhsT(ki, mlo) for ki in range(2)]
                # colsum PSUM tiles: blocks land at base partitions
                # 0/32/64/96 so one strided copy drains 4 blocks at once
                csA = cs_pool.tile([P, 1024], F32, tag="csA")
                stage = cst_pool.tile([1, CPW], F32, tag="cstage")
                nc.vector.memset(stage[0:1, 0:P], 0.0)
                for pair in range(4):
                    ps = ps_pool.tile([P, 1024], F32, tag="ps")
                    for ki in range(2):
                        for b2 in range(2):
                            c0 = mlo + 1024 * pair + 512 * b2
                            nc.tensor.matmul(
                                ps[:, 512 * b2:512 * (b2 + 1)],
                                lhsT=lhsT[ki], rhs=dr_view(ki, c0, 512),
                                start=(ki == 0), stop=(ki == 1), perf_mode=DR)
                    e_scr = e_pool.tile([P, 1024], F8, tag="escr")
                    nc.scalar.activation(
                        out=e_scr, in_=ps, func=AF.Exp, scale=EXP_SCALE,
                        accum_out=rowacc[:, pair:pair + 1])
                    for b2 in range(2):
                        blk = 2 * pair + b2          # 0..7
                        # quadrant row 32*(blk//2) holds blocks 2k, 2k+1
                        co = 512 * (blk % 2)
                        bp = 32 * (blk // 2)
                        lo = P if blk == 0 else 0
                        nc.tensor.matmul(
                            csA[bp:bp + 1, co + lo:co + 512],
                            lhsT=ones8[:, 0:1],
                            rhs=e_scr[:, 512 * b2 + lo:512 * (b2 + 1)],
                            start=True, stop=True, tile_position=(0, bp))
                # drain colsums: one copy per PE quadrant row (DVE+ACT split)
                nc.vector.tensor_scalar_add(
                    out=stage[0:1, P:1024], in0=csA[0:1, P:1024], scalar1=0.0)
                nc.vector.tensor_scalar_add(
                    out=stage[0:1, 1024:2048], in0=csA[32:33, 0:1024], scalar1=0.0)
                nc.scalar.copy(out=stage[0:1, 2048:3072], in_=csA[64:65, 0:1024])
                nc.scalar.copy(out=stage[0:1, 3072:4096], in_=csA[96:97, 0:1024])
                if t < 4:
                    ps = ps_pool.tile([P, 1024], F32, tag="ps")
                    for ki in range(2):
                        nc.tensor.matmul(
                            ps[:, 0:P], lhsT=lhsT[ki],
                            rhs=dr_view(ki, mlo + 4096, P),
                            start=(ki == 0), stop=(ki == 1), perf_mode=DR)
                    e32 = e32_pool.tile([P, P], BF16, tag="e32")
                    nc.scalar.activation(
                        out=e32, in_=ps[:, 0:P], func=AF.Exp, scale=EXP_SCALE,
                        accum_out=rowacc[:, 4:5])
                    # j32 colsum reuses the unused half of the j32 S psum tile
                    nc.tensor.matmul(
                        ps[0:1, 512:512 + P], lhsT=ones16[:, 0:1], rhs=e32,
                        start=True, stop=True)
                    nc.vector.tensor_scalar_add(
                        out=stage[0:1, 4096:CPW], in0=ps[0:1, 512:512 + P],
                        scalar1=0.0)
                    dsel = e32_pool.tile([P, P], BF16, tag="dsel")
                    nc.gpsimd.affine_select(
                        out=dsel, in_=e32, pattern=[[1, P]],
                        compare_op=ALU.is_equal, fill=0.0,
                        base=0, channel_multiplier=-1)
                    nc.vector.reduce_sum(
                        out=posd_sb[:, t:t + 1], in_=dsel,
                        axis=mybir.AxisListType.X)
                else:
                    nc.vector.memset(rowacc[:, 4:5], 0.0)
                nc.vector.reduce_sum(
                    out=rowp_sb[:, t:t + 1], in_=rowacc,
                    axis=mybir.AxisListType.X)
                kmax = CPW if t < 4 else 4096
                nc.sync.dma_start(
                    out=colp_out[t:t + 1, 0:kmax], in_=stage[0:1, 0:kmax])

            nc.sync.dma_start(out=rowp_out[:, :], in_=rowp_sb)
            nc.sync.dma_start(out=posd_out[:, :], in_=posd_sb)

    nc.finalize()
    return nc


_CACHE = {}


def _run(full: np.ndarray, trace: bool = False, **kwargs):
    """Run the SPMD program on all 8 cores; returns BassKernelResults."""
    if "nc" not in _CACHE:
        _CACHE["nc"] = _build_program()
    nc = _CACHE["nc"]
    in_maps = [
        {"x": np.ascontiguousarray(np.roll(full, -P * c, axis=0))}
        for c in range(N_CORES)
    ]
    return run_bass_kernel_spmd(
        nc, in_maps, core_ids=list(range(N_CORES)), trace=trace, **kwargs)


def _merge(results) -> np.ndarray:
    denom = np.zeros(ROWS, np.float64)
    pos = np.zeros(BATCH, np.float64)
    idx = np.arange(P)
    for c, r in enumerate(results):
        rowp = r["rowp"].astype(np.float64)
        colp = r["colp"].astype(np.float64)
        posd = r["posd"].astype(np.float64)
        for t in range(8):
            gr = (1024 * t + idx + P * c) % ROWS
            denom[gr] += rowp[:, t]
            kmax = CPW if t < 4 else 4096
            k = np.arange(P, kmax)
            gc = (1024 * t + k + P * c) % ROWS
            np.add.at(denom, gc, colp[t, k])
            if t < 4:
                pos[(1024 * t + idx + P * c) % ROWS] = posd[:, t]
    denom -= np.exp(2.0)
    loss = (np.log(denom).sum() - 2.0 * np.log(pos).sum()) / ROWS
    return np.array(loss, dtype=np.float32)


def kernel(emb_i: np.ndarray, emb_j: np.ndarray) -> np.ndarray:
    full = np.concatenate(
        [np.asarray(emb_i, np.float32), np.asarray(emb_j, np.float32)], axis=0)
    return _merge(_run(full).results)
